# revision 3
# baseline (speedup 1.0000x reference)
"""Trainium2 Bass kernel for nn_DenseAttentionOneHead (B=2, L=4096, H=1024).

Reference math:
    h   = hidden * cos + rotate_half(hidden) * sin      (RoPE)
    q   = h @ W_q.T
    out = (q @ h^T) @ h                                 (no softmax)

With no softmax the L x L score matrix factorizes away:
    out[b] = q[b] @ G[b],  G[b] = h[b].T @ h[b]  (H x H)
reducing the work from O(B L^2 H) to O(B L H^2) ~ 39 GFLOP total.

Sharding (8 NeuronCores): cores 0-3 own batch 0's four 1024-row L-chunks,
cores 4-7 batch 1. Each core computes a partial G over its chunk; two 2MB
AllReduces within each 4-core replica group (pipelined against the q^T
matmul) produce the full G. q^T comes from PE-transposed RoPE output so
no transposed operands are ever loaded from HBM. All matmuls run in
float32r (TF32-like ~13-bit mantissa, full PE rate at free-dim 512, rel
err ~1.5e-4); RoPE, PSUM accumulation and the AllReduce stay fp32.

Engine plan per core: DVE does RoPE + fp32r roundings; PE does 64
transposes (filling its stalls while RoPE streams) then 3 x 128 matmuls;
ACT does all PSUM->SBUF copies; the two HWDGE DMA rings are split so the
W_q load / G fetch never queue behind the input stream or G bounce-out.
"""

import os

import numpy as np

import jax

try:
    _cache_dir = os.path.join(os.path.expanduser("~"), ".cache", "bass_kernel_jax")
    os.makedirs(_cache_dir, exist_ok=True)
    jax.config.update("jax_compilation_cache_dir", _cache_dir)
    jax.config.update("jax_persistent_cache_min_compile_time_secs", 1.0)
except Exception:
    pass

import concourse.bacc as bacc
import concourse.mybir as mybir
import concourse.tile as tile
from concourse import masks
from concourse.bass_utils import run_bass_kernel_spmd

F32 = mybir.dt.float32
F32R = mybir.dt.float32r

B, L, H = 2, 4096, 1024
L_CHUNK = 1024
HH = H // 2
NT = L_CHUNK // 128
MT = H // 128
GROUPS = [[0, 1, 2, 3], [4, 5, 6, 7]]


def _emit_once(nc, tc, h_d, c1_d, s1_d, wqt_d, y_d):
    h_ap = h_d.ap().rearrange("(t p) c -> p t c", p=128)
    c1_ap = c1_d.ap().rearrange("(t p) c -> p t c", p=128)
    s1_ap = s1_d.ap().rearrange("(t p) c -> p t c", p=128)
    wqt_ap = wqt_d.ap().rearrange("(t p) c -> p t c", p=128)
    y_ap = y_d.ap().rearrange("(t p) c -> p t c", p=128)

    with (
        tc.tile_pool(name="persist", bufs=1) as persist,
        tc.tile_pool(name="stream", bufs=1) as stream,
        tc.tile_pool(name="psum", bufs=6, space="PSUM") as psum,
        tc.tile_pool(name="psum_t", bufs=2, space="PSUM") as psum_t,
        tc.tile_pool(name="dram", bufs=1, space="DRAM") as dram,
    ):
        hr = persist.tile([128, NT, H], F32R, name="hr")
        hrt = persist.tile([128, MT, L_CHUNK], F32R, name="hrt")
        wq_r = persist.tile([128, MT, H], F32R, name="wq_r")
        # qt reuses hr's memory: hr's last readers (G matmuls + transposes)
        # finish right before the qt copies start writing; Tile WAR deps
        # order them.
        qt = hr
        g_r = persist.tile([128, MT, H], F32R, name="g_r")

        ident_f = stream.tile([128, 128], F32, name="ident_f", tag="identf")
        masks.make_identity(nc, ident_f[:])
        ident = stream.tile([128, 128], F32R, name="ident", tag="ident")
        nc.vector.tensor_copy(ident[:], ident_f[:])

        # W_qT load on the ACT HWDGE ring; rounded on DVE after RoPE.
        wq_stage = []
        for mt in range(MT):
            wt = stream.tile([128, H], F32, name="wt", tag="ldw", bufs=2)
            nc.scalar.dma_start(wt[:], wqt_ap[:, mt, :])
            wq_stage.append(wt)

        # RoPE (DVE) + per-tile PE transposes
        for t in range(NT):
            ht = stream.tile([128, H], F32, name="ht", tag="ld1024", bufs=3)
            ct = stream.tile([128, HH], F32, name="ct", tag="ld512", bufs=4)
            st = stream.tile([128, HH], F32, name="st", tag="ld512", bufs=4)
            nc.sync.dma_start(ht[:], h_ap[:, t, :])
            nc.sync.dma_start(ct[:], c1_ap[:, t, :])
            nc.sync.dma_start(st[:], s1_ap[:, t, :])
            h1 = ht[:, 0:HH]
            h2 = ht[:, HH:H]
            m1 = stream.tile([128, HH], F32, name="m1", tag="tmp", bufs=4)
            m2 = stream.tile([128, HH], F32, name="m2", tag="tmp", bufs=4)
            nc.vector.tensor_mul(m1[:], h1, ct[:])
            nc.vector.tensor_mul(m2[:], h2, st[:])
            nc.vector.tensor_sub(hr[:, t, 0:HH], m1[:], m2[:])
            m3 = stream.tile([128, HH], F32, name="m3", tag="tmp", bufs=4)
            m4 = stream.tile([128, HH], F32, name="m4", tag="tmp", bufs=4)
            nc.vector.tensor_mul(m3[:], h2, ct[:])
            nc.vector.tensor_mul(m4[:], h1, st[:])
            nc.vector.tensor_add(hr[:, t, HH:H], m3[:], m4[:])
            for mt in range(MT):
                pst = psum_t.tile([128, 128], F32R, name="pst", tag="pst")
                nc.tensor.transpose(
                    pst[:], hr[:, t, mt * 128:(mt + 1) * 128], ident[:]
                )
                nc.scalar.copy(hrt[:, mt, t * 128:(t + 1) * 128], pst[:])

        for mt in range(MT):
            nc.vector.tensor_copy(wq_r[:, mt, :], wq_stage[mt][:])

        # G_part = hr.T @ hr; bounce per m-tile; AllReduce each 4-row half
        halves_in = [
            dram.tile([128, 4 * H], F32, name=f"bounce_in{i}", tag=f"bi{i}")
            for i in range(2)
        ]
        halves_out = [
            dram.tile([128, 4 * H], F32, name=f"bounce_out{i}", tag=f"bo{i}")
            for i in range(2)
        ]
        b_in_t = [h[:].rearrange("p (t c) -> p t c", t=4) for h in halves_in]
        b_out_t = [h[:].rearrange("p (t c) -> p t c", t=4) for h in halves_out]
        for piece in range(2):
            for mt in range(piece * 4, piece * 4 + 4):
                gb = stream.tile([128, H], F32, name="gb", tag="gb", bufs=2)
                for nh in range(2):
                    ps = psum.tile([128, 512], F32, name="ps", tag="ps")
                    for kt in range(NT):
                        nc.tensor.matmul(
                            ps[:],
                            hr[:, kt, mt * 128:(mt + 1) * 128],
                            hr[:, kt, nh * 512:(nh + 1) * 512],
                            start=(kt == 0),
                            stop=(kt == NT - 1),
                        )
                    nc.scalar.copy(gb[:, nh * 512:(nh + 1) * 512], ps[:])
                nc.sync.dma_start(b_in_t[piece][:, mt - piece * 4, :], gb[:])
            nc.gpsimd.collective_compute(
                "AllReduce",
                mybir.AluOpType.add,
                replica_groups=GROUPS,
                ins=[halves_in[piece][:]],
                outs=[halves_out[piece][:]],
            )

        # qt = (W_qT as weights) @ hrt
        for ot in range(MT):
            for lh in range(2):
                ps = psum.tile([128, 512], F32, name="ps", tag="ps")
                for kt in range(MT):
                    nc.tensor.matmul(
                        ps[:],
                        wq_r[:, kt, ot * 128:(ot + 1) * 128],
                        hrt[:, kt, lh * 512:(lh + 1) * 512],
                        start=(kt == 0),
                        stop=(kt == MT - 1),
                    )
                nc.scalar.copy(qt[:, ot, lh * 512:(lh + 1) * 512], ps[:])

        # fetch AR halves on ACT ring as they land; round on DVE
        for mt in range(MT):
            gi = stream.tile([128, H], F32, name="gi", tag="gi", bufs=2)
            nc.scalar.dma_start(gi[:], b_out_t[mt // 4][:, mt % 4, :])
            nc.vector.tensor_copy(g_r[:, mt, :], gi[:])

        # y = (qt as weights) @ G
        for lt in range(NT):
            for nh in range(2):
                ps = psum.tile([128, 512], F32, name="ps", tag="ps")
                for kt in range(MT):
                    nc.tensor.matmul(
                        ps[:],
                        qt[:, kt, lt * 128:(lt + 1) * 128],
                        g_r[:, kt, nh * 512:(nh + 1) * 512],
                        start=(kt == 0),
                        stop=(kt == MT - 1),
                    )
                ot = stream.tile([128, 512], F32, name="ot", tag="ld512", bufs=4)
                nc.scalar.copy(ot[:], ps[:])
                nc.sync.dma_start(y_ap[:, lt, nh * 512:(nh + 1) * 512], ot[:])


_NC_CACHE = {}


def _build():
    if "nc" in _NC_CACHE:
        return _NC_CACHE["nc"]
    nc = bacc.Bacc("TRN2", target_bir_lowering=False, debug=False, num_devices=8)
    h_d = nc.dram_tensor("h", [L_CHUNK, H], F32, kind="ExternalInput")
    c1_d = nc.dram_tensor("c1", [L_CHUNK, HH], F32, kind="ExternalInput")
    s1_d = nc.dram_tensor("s1", [L_CHUNK, HH], F32, kind="ExternalInput")
    wqt_d = nc.dram_tensor("wqt", [H, H], F32, kind="ExternalInput")
    y_d = nc.dram_tensor("y", [L_CHUNK, H], F32, kind="ExternalOutput")
    with tile.TileContext(nc) as tc:
        _emit_once(nc, tc, h_d, c1_d, s1_d, wqt_d, y_d)
    nc.compile()
    _NC_CACHE["nc"] = nc
    return nc


def kernel(hidden_states, W_q, cos, sin):
    hs = np.asarray(hidden_states, dtype=np.float32)
    wq = np.asarray(W_q, dtype=np.float32)
    cos = np.asarray(cos, dtype=np.float32)
    sin = np.asarray(sin, dtype=np.float32)
    wqt = np.ascontiguousarray(wq.T)
    in_maps = []
    for core in range(8):
        b, i = core // 4, core % 4
        sl = slice(i * L_CHUNK, (i + 1) * L_CHUNK)
        in_maps.append({
            "h": np.ascontiguousarray(hs[b, sl]),
            "c1": np.ascontiguousarray(cos[sl, :HH]),
            "s1": np.ascontiguousarray(sin[sl, :HH]),
            "wqt": wqt,
        })

    nc = _build()
    res = run_bass_kernel_spmd(nc, in_maps, core_ids=list(range(8)))

    out = np.empty((B, L, H), dtype=np.float32)
    for core, r in enumerate(res.results):
        b, i = core // 4, core % 4
        out[b, i * L_CHUNK:(i + 1) * L_CHUNK] = r["y"]
    return out


# revision 4
# speedup vs baseline: 98.9512x; 98.9512x over previous
"""Trainium2 Bass kernel for nn_DenseAttentionOneHead (B=2, L=4096, H=1024).

Reference math:
    h   = hidden * cos + rotate_half(hidden) * sin      (RoPE)
    q   = h @ W_q.T
    out = (q @ h^T) @ h                                 (no softmax)

With no softmax the L x L score matrix factorizes away:
    out[b] = q[b] @ G[b],  G[b] = h[b].T @ h[b]  (H x H)
reducing the work from O(B L^2 H) to O(B L H^2) ~ 39 GFLOP total.

Sharding (8 NeuronCores): cores 0-3 own batch 0's four 1024-row L-chunks,
cores 4-7 batch 1. Each core computes a partial G over its chunk; two 2MB
AllReduces within each 4-core replica group (pipelined against the q^T
matmul) produce the full G. q^T comes from PE-transposed RoPE output so
no transposed operands are ever loaded from HBM. All matmuls run in
float32r (TF32-like ~13-bit mantissa, full PE rate at free-dim 512, rel
err ~1.5e-4); RoPE, PSUM accumulation and the AllReduce stay fp32.

Engine plan per core: DVE does RoPE + fp32r roundings; PE does 64
transposes (filling its stalls while RoPE streams) then 3 x 128 matmuls;
ACT does all PSUM->SBUF copies; the two HWDGE DMA rings are split so the
W_q load / G fetch never queue behind the input stream or G bounce-out.
"""

import os

import numpy as np

import jax

try:
    _cache_dir = os.path.join(os.path.expanduser("~"), ".cache", "bass_kernel_jax")
    os.makedirs(_cache_dir, exist_ok=True)
    jax.config.update("jax_compilation_cache_dir", _cache_dir)
    jax.config.update("jax_persistent_cache_min_compile_time_secs", 1.0)
except Exception:
    pass

import concourse.bacc as bacc
import concourse.mybir as mybir
import concourse.tile as tile
from concourse import masks
from concourse.bass_utils import run_bass_kernel_spmd

F32 = mybir.dt.float32
F32R = mybir.dt.float32r

B, L, H = 2, 4096, 1024
L_CHUNK = 1024
HH = H // 2
NT = L_CHUNK // 128
MT = H // 128
GROUPS = [[0, 1, 2, 3], [4, 5, 6, 7]]


def _emit_once(nc, tc, h_d, c1_d, s1_d, wqt_d, y_d):
    h_ap = h_d.ap().rearrange("(t p) c -> p t c", p=128)
    c1_ap = c1_d.ap().rearrange("(t p) c -> p t c", p=128)
    s1_ap = s1_d.ap().rearrange("(t p) c -> p t c", p=128)
    wqt_ap = wqt_d.ap().rearrange("(t p) c -> p t c", p=128)
    y_ap = y_d.ap().rearrange("(t p) c -> p t c", p=128)

    with (
        tc.tile_pool(name="persist", bufs=1) as persist,
        tc.tile_pool(name="stream", bufs=1) as stream,
        tc.tile_pool(name="psum", bufs=6, space="PSUM") as psum,
        tc.tile_pool(name="psum_t", bufs=2, space="PSUM") as psum_t,
        tc.tile_pool(name="dram", bufs=1, space="DRAM") as dram,
    ):
        hr = persist.tile([128, NT, H], F32R, name="hr")
        hrt = persist.tile([128, MT, L_CHUNK], F32R, name="hrt")
        wq_r = persist.tile([128, MT, H], F32R, name="wq_r")
        # qt reuses hr's memory: hr's last readers (G matmuls + transposes)
        # finish right before the qt copies start writing; Tile WAR deps
        # order them.
        qt = hr
        g_r = persist.tile([128, MT, H], F32R, name="g_r")

        ident_f = stream.tile([128, 128], F32, name="ident_f", tag="identf")
        masks.make_identity(nc, ident_f[:])
        ident = stream.tile([128, 128], F32R, name="ident", tag="ident")
        nc.vector.tensor_copy(ident[:], ident_f[:])

        # W_qT load on the ACT HWDGE ring; rounded on DVE after RoPE.
        wq_stage = []
        for mt in range(MT):
            wt = stream.tile([128, H], F32, name="wt", tag="ldw", bufs=2)
            nc.scalar.dma_start(wt[:], wqt_ap[:, mt, :])
            wq_stage.append(wt)

        # RoPE (DVE) + per-tile PE transposes
        for t in range(NT):
            ht = stream.tile([128, H], F32, name="ht", tag="ld1024", bufs=3)
            ct = stream.tile([128, HH], F32, name="ct", tag="ld512", bufs=4)
            st = stream.tile([128, HH], F32, name="st", tag="ld512", bufs=4)
            nc.sync.dma_start(ht[:], h_ap[:, t, :])
            nc.sync.dma_start(ct[:], c1_ap[:, t, :])
            nc.sync.dma_start(st[:], s1_ap[:, t, :])
            h1 = ht[:, 0:HH]
            h2 = ht[:, HH:H]
            m1 = stream.tile([128, HH], F32, name="m1", tag="tmp", bufs=4)
            m2 = stream.tile([128, HH], F32, name="m2", tag="tmp", bufs=4)
            nc.vector.tensor_mul(m1[:], h1, ct[:])
            nc.vector.tensor_mul(m2[:], h2, st[:])
            nc.vector.tensor_sub(hr[:, t, 0:HH], m1[:], m2[:])
            m3 = stream.tile([128, HH], F32, name="m3", tag="tmp", bufs=4)
            m4 = stream.tile([128, HH], F32, name="m4", tag="tmp", bufs=4)
            nc.vector.tensor_mul(m3[:], h2, ct[:])
            nc.vector.tensor_mul(m4[:], h1, st[:])
            nc.vector.tensor_add(hr[:, t, HH:H], m3[:], m4[:])
            for mt in range(MT):
                pst = psum_t.tile([128, 128], F32R, name="pst", tag="pst")
                nc.tensor.transpose(
                    pst[:], hr[:, t, mt * 128:(mt + 1) * 128], ident[:]
                )
                nc.scalar.copy(hrt[:, mt, t * 128:(t + 1) * 128], pst[:])

        for mt in range(MT):
            nc.vector.tensor_copy(wq_r[:, mt, :], wq_stage[mt][:])

        # G_part = hr.T @ hr; bounce per m-tile; AllReduce each 4-row half
        bounce_in = dram.tile([128, MT * H], F32, name="bounce_in")
        bounce_out = dram.tile([128, MT * H], F32, name="bounce_out")
        b_in_t = bounce_in[:].rearrange("p (t c) -> p t c", t=MT)
        b_out_t = bounce_out[:].rearrange("p (t c) -> p t c", t=MT)
        for mt in range(MT):
            gb = stream.tile([128, H], F32, name="gb", tag="gb", bufs=2)
            for nh in range(2):
                ps = psum.tile([128, 512], F32, name="ps", tag="ps")
                for kt in range(NT):
                    nc.tensor.matmul(
                        ps[:],
                        hr[:, kt, mt * 128:(mt + 1) * 128],
                        hr[:, kt, nh * 512:(nh + 1) * 512],
                        start=(kt == 0),
                        stop=(kt == NT - 1),
                    )
                nc.scalar.copy(gb[:, nh * 512:(nh + 1) * 512], ps[:])
            nc.sync.dma_start(b_in_t[:, mt, :], gb[:])
        # single AllReduce: per-collective fixed cost dominates on this
        # fabric path, so one 4MB AR beats two pipelined 2MB ARs
        nc.gpsimd.collective_compute(
            "AllReduce",
            mybir.AluOpType.add,
            replica_groups=GROUPS,
            ins=[bounce_in[:]],
            outs=[bounce_out[:]],
        )

        # qt = (W_qT as weights) @ hrt
        for ot in range(MT):
            for lh in range(2):
                ps = psum.tile([128, 512], F32, name="ps", tag="ps")
                for kt in range(MT):
                    nc.tensor.matmul(
                        ps[:],
                        wq_r[:, kt, ot * 128:(ot + 1) * 128],
                        hrt[:, kt, lh * 512:(lh + 1) * 512],
                        start=(kt == 0),
                        stop=(kt == MT - 1),
                    )
                nc.scalar.copy(qt[:, ot, lh * 512:(lh + 1) * 512], ps[:])

        # fetch AR halves on ACT ring as they land; round on DVE
        for mt in range(MT):
            gi = stream.tile([128, H], F32, name="gi", tag="gi", bufs=2)
            nc.scalar.dma_start(gi[:], b_out_t[:, mt, :])
            nc.vector.tensor_copy(g_r[:, mt, :], gi[:])

        # y = (qt as weights) @ G
        for lt in range(NT):
            for nh in range(2):
                ps = psum.tile([128, 512], F32, name="ps", tag="ps")
                for kt in range(MT):
                    nc.tensor.matmul(
                        ps[:],
                        qt[:, kt, lt * 128:(lt + 1) * 128],
                        g_r[:, kt, nh * 512:(nh + 1) * 512],
                        start=(kt == 0),
                        stop=(kt == MT - 1),
                    )
                ot = stream.tile([128, 512], F32, name="ot", tag="ld512", bufs=4)
                nc.scalar.copy(ot[:], ps[:])
                nc.sync.dma_start(y_ap[:, lt, nh * 512:(nh + 1) * 512], ot[:])


_NC_CACHE = {}


def _build():
    if "nc" in _NC_CACHE:
        return _NC_CACHE["nc"]
    nc = bacc.Bacc("TRN2", target_bir_lowering=False, debug=False, num_devices=8)
    h_d = nc.dram_tensor("h", [L_CHUNK, H], F32, kind="ExternalInput")
    c1_d = nc.dram_tensor("c1", [L_CHUNK, HH], F32, kind="ExternalInput")
    s1_d = nc.dram_tensor("s1", [L_CHUNK, HH], F32, kind="ExternalInput")
    wqt_d = nc.dram_tensor("wqt", [H, H], F32, kind="ExternalInput")
    y_d = nc.dram_tensor("y", [L_CHUNK, H], F32, kind="ExternalOutput")
    with tile.TileContext(nc) as tc:
        _emit_once(nc, tc, h_d, c1_d, s1_d, wqt_d, y_d)
    nc.compile()
    _NC_CACHE["nc"] = nc
    return nc


def kernel(hidden_states, W_q, cos, sin):
    hs = np.asarray(hidden_states, dtype=np.float32)
    wq = np.asarray(W_q, dtype=np.float32)
    cos = np.asarray(cos, dtype=np.float32)
    sin = np.asarray(sin, dtype=np.float32)
    wqt = np.ascontiguousarray(wq.T)
    in_maps = []
    for core in range(8):
        b, i = core // 4, core % 4
        sl = slice(i * L_CHUNK, (i + 1) * L_CHUNK)
        in_maps.append({
            "h": np.ascontiguousarray(hs[b, sl]),
            "c1": np.ascontiguousarray(cos[sl, :HH]),
            "s1": np.ascontiguousarray(sin[sl, :HH]),
            "wqt": wqt,
        })

    nc = _build()
    res = run_bass_kernel_spmd(nc, in_maps, core_ids=list(range(8)))

    out = np.empty((B, L, H), dtype=np.float32)
    for core, r in enumerate(res.results):
        b, i = core // 4, core % 4
        out[b, i * L_CHUNK:(i + 1) * L_CHUNK] = r["y"]
    return out


# revision 6
# speedup vs baseline: 100.9546x; 1.0202x over previous
"""Trainium2 Bass kernel for nn_DenseAttentionOneHead (B=2, L=4096, H=1024).

Reference math:
    h   = hidden * cos + rotate_half(hidden) * sin      (RoPE)
    q   = h @ W_q.T
    out = (q @ h^T) @ h                                 (no softmax)

With no softmax the L x L score matrix factorizes away:
    out[b] = q[b] @ G[b],  G[b] = h[b].T @ h[b]  (H x H)
reducing the work from O(B L^2 H) to O(B L H^2) ~ 39 GFLOP total.

Sharding (8 NeuronCores): cores 0-3 own batch 0's four 1024-row L-chunks,
cores 4-7 batch 1. Each core computes a partial G over its chunk; two 2MB
AllReduces within each 4-core replica group (pipelined against the q^T
matmul) produce the full G. q^T comes from PE-transposed RoPE output so
no transposed operands are ever loaded from HBM. All matmuls run in
float32r (TF32-like ~13-bit mantissa, full PE rate at free-dim 512, rel
err ~1.5e-4); RoPE, PSUM accumulation and the AllReduce stay fp32.

Engine plan per core: DVE does RoPE + fp32r roundings; PE does 64
transposes (filling its stalls while RoPE streams) then 3 x 128 matmuls;
ACT does all PSUM->SBUF copies; the two HWDGE DMA rings are split (h/G
bounce/y on the SP ring; cos/sin/W_q/G-fetch on the ACT ring) so the h
stream and the G bounce-out are never queued behind other transfers.
"""

import os

import numpy as np

import jax

try:
    _cache_dir = os.path.join(os.path.expanduser("~"), ".cache", "bass_kernel_jax")
    os.makedirs(_cache_dir, exist_ok=True)
    jax.config.update("jax_compilation_cache_dir", _cache_dir)
    jax.config.update("jax_persistent_cache_min_compile_time_secs", 1.0)
except Exception:
    pass

import concourse.bacc as bacc
import concourse.mybir as mybir
import concourse.tile as tile
from concourse import masks
from concourse.bass_utils import run_bass_kernel_spmd

F32 = mybir.dt.float32
F32R = mybir.dt.float32r

B, L, H = 2, 4096, 1024
L_CHUNK = 1024
HH = H // 2
NT = L_CHUNK // 128
MT = H // 128
GROUPS = [[0, 1, 2, 3], [4, 5, 6, 7]]


def _emit_once(nc, tc, h_d, c1_d, s1_d, wqt_d, y_d):
    h_ap = h_d.ap().rearrange("(t p) c -> p t c", p=128)
    c1_ap = c1_d.ap().rearrange("(t p) c -> p t c", p=128)
    s1_ap = s1_d.ap().rearrange("(t p) c -> p t c", p=128)
    wqt_ap = wqt_d.ap().rearrange("(t p) c -> p t c", p=128)
    y_ap = y_d.ap().rearrange("(t p) c -> p t c", p=128)

    with (
        tc.tile_pool(name="persist", bufs=1) as persist,
        tc.tile_pool(name="stream", bufs=1) as stream,
        tc.tile_pool(name="psum", bufs=6, space="PSUM") as psum,
        tc.tile_pool(name="psum_t", bufs=2, space="PSUM") as psum_t,
        tc.tile_pool(name="dram", bufs=1, space="DRAM") as dram,
    ):
        hr = persist.tile([128, NT, H], F32R, name="hr")
        hrt = persist.tile([128, MT, L_CHUNK], F32R, name="hrt")
        wq_r = persist.tile([128, MT, H], F32R, name="wq_r")
        # qt reuses hr's memory: hr's last readers (G matmuls + transposes)
        # finish right before the qt copies start writing; Tile WAR deps
        # order them.
        qt = hr
        g_r = persist.tile([128, MT, H], F32R, name="g_r")

        ident_f = stream.tile([128, 128], F32, name="ident_f", tag="identf")
        masks.make_identity(nc, ident_f[:])
        ident = stream.tile([128, 128], F32R, name="ident", tag="ident")
        nc.vector.tensor_copy(ident[:], ident_f[:])

        # RoPE (DVE) + per-tile PE transposes
        for t in range(NT):
            ht = stream.tile([128, H], F32, name="ht", tag="ld1024", bufs=3)
            ct = stream.tile([128, HH], F32, name="ct", tag="ld512", bufs=4)
            st = stream.tile([128, HH], F32, name="st", tag="ld512", bufs=4)
            nc.sync.dma_start(ht[:], h_ap[:, t, :])
            nc.scalar.dma_start(ct[:], c1_ap[:, t, :])
            nc.scalar.dma_start(st[:], s1_ap[:, t, :])
            h1 = ht[:, 0:HH]
            h2 = ht[:, HH:H]
            m1 = stream.tile([128, HH], F32, name="m1", tag="tmp", bufs=4)
            m2 = stream.tile([128, HH], F32, name="m2", tag="tmp", bufs=4)
            nc.vector.tensor_mul(m1[:], h1, ct[:])
            nc.vector.tensor_mul(m2[:], h2, st[:])
            nc.vector.tensor_sub(hr[:, t, 0:HH], m1[:], m2[:])
            m3 = stream.tile([128, HH], F32, name="m3", tag="tmp", bufs=4)
            m4 = stream.tile([128, HH], F32, name="m4", tag="tmp", bufs=4)
            nc.vector.tensor_mul(m3[:], h2, ct[:])
            nc.vector.tensor_mul(m4[:], h1, st[:])
            nc.vector.tensor_add(hr[:, t, HH:H], m3[:], m4[:])
            for mt in range(MT):
                pst = psum_t.tile([128, 128], F32R, name="pst", tag="pst")
                nc.tensor.transpose(
                    pst[:], hr[:, t, mt * 128:(mt + 1) * 128], ident[:]
                )
                nc.scalar.copy(hrt[:, mt, t * 128:(t + 1) * 128], pst[:])

        # W_qT load (ACT ring, queued behind the c/s stream) + DVE round
        for mt in range(MT):
            wt = stream.tile([128, H], F32, name="wt", tag="ldw", bufs=2)
            nc.scalar.dma_start(wt[:], wqt_ap[:, mt, :])
            nc.vector.tensor_copy(wq_r[:, mt, :], wt[:])

        # G_part = hr.T @ hr; bounce per m-tile; AllReduce each 4-row half
        bounce_in = dram.tile([128, MT * H], F32, name="bounce_in")
        bounce_out = dram.tile([128, MT * H], F32, name="bounce_out")
        b_in_t = bounce_in[:].rearrange("p (t c) -> p t c", t=MT)
        b_out_t = bounce_out[:].rearrange("p (t c) -> p t c", t=MT)
        for mt in range(MT):
            gb = stream.tile([128, H], F32, name="gb", tag="gb", bufs=2)
            for nh in range(2):
                ps = psum.tile([128, 512], F32, name="ps", tag="ps")
                for kt in range(NT):
                    nc.tensor.matmul(
                        ps[:],
                        hr[:, kt, mt * 128:(mt + 1) * 128],
                        hr[:, kt, nh * 512:(nh + 1) * 512],
                        start=(kt == 0),
                        stop=(kt == NT - 1),
                    )
                nc.scalar.copy(gb[:, nh * 512:(nh + 1) * 512], ps[:])
            nc.sync.dma_start(b_in_t[:, mt, :], gb[:])
        # single AllReduce: per-collective fixed cost dominates on this
        # fabric path, so one 4MB AR beats two pipelined 2MB ARs
        nc.gpsimd.collective_compute(
            "AllReduce",
            mybir.AluOpType.add,
            replica_groups=GROUPS,
            ins=[bounce_in[:]],
            outs=[bounce_out[:]],
        )

        # qt = (W_qT as weights) @ hrt
        for ot in range(MT):
            for lh in range(2):
                ps = psum.tile([128, 512], F32, name="ps", tag="ps")
                for kt in range(MT):
                    nc.tensor.matmul(
                        ps[:],
                        wq_r[:, kt, ot * 128:(ot + 1) * 128],
                        hrt[:, kt, lh * 512:(lh + 1) * 512],
                        start=(kt == 0),
                        stop=(kt == MT - 1),
                    )
                nc.scalar.copy(qt[:, ot, lh * 512:(lh + 1) * 512], ps[:])

        # fetch AllReduce result on the ACT ring; round to fp32r on DVE
        for mt in range(MT):
            gi = stream.tile([128, H], F32, name="gi", tag="gi", bufs=2)
            nc.scalar.dma_start(gi[:], b_out_t[:, mt, :])
            nc.vector.tensor_copy(g_r[:, mt, :], gi[:])

        # y = (qt as weights) @ G
        for lt in range(NT):
            for nh in range(2):
                ps = psum.tile([128, 512], F32, name="ps", tag="ps")
                for kt in range(MT):
                    nc.tensor.matmul(
                        ps[:],
                        qt[:, kt, lt * 128:(lt + 1) * 128],
                        g_r[:, kt, nh * 512:(nh + 1) * 512],
                        start=(kt == 0),
                        stop=(kt == MT - 1),
                    )
                ot = stream.tile([128, 512], F32, name="ot", tag="ld512", bufs=4)
                nc.scalar.copy(ot[:], ps[:])
                nc.sync.dma_start(y_ap[:, lt, nh * 512:(nh + 1) * 512], ot[:])


_NC_CACHE = {}


def _build():
    if "nc" in _NC_CACHE:
        return _NC_CACHE["nc"]
    nc = bacc.Bacc("TRN2", target_bir_lowering=False, debug=False, num_devices=8)
    h_d = nc.dram_tensor("h", [L_CHUNK, H], F32, kind="ExternalInput")
    c1_d = nc.dram_tensor("c1", [L_CHUNK, HH], F32, kind="ExternalInput")
    s1_d = nc.dram_tensor("s1", [L_CHUNK, HH], F32, kind="ExternalInput")
    wqt_d = nc.dram_tensor("wqt", [H, H], F32, kind="ExternalInput")
    y_d = nc.dram_tensor("y", [L_CHUNK, H], F32, kind="ExternalOutput")
    with tile.TileContext(nc) as tc:
        _emit_once(nc, tc, h_d, c1_d, s1_d, wqt_d, y_d)
    nc.compile()
    _NC_CACHE["nc"] = nc
    return nc


def kernel(hidden_states, W_q, cos, sin):
    hs = np.asarray(hidden_states, dtype=np.float32)
    wq = np.asarray(W_q, dtype=np.float32)
    cos = np.asarray(cos, dtype=np.float32)
    sin = np.asarray(sin, dtype=np.float32)
    wqt = np.ascontiguousarray(wq.T)
    in_maps = []
    for core in range(8):
        b, i = core // 4, core % 4
        sl = slice(i * L_CHUNK, (i + 1) * L_CHUNK)
        in_maps.append({
            "h": np.ascontiguousarray(hs[b, sl]),
            "c1": np.ascontiguousarray(cos[sl, :HH]),
            "s1": np.ascontiguousarray(sin[sl, :HH]),
            "wqt": wqt,
        })

    nc = _build()
    res = run_bass_kernel_spmd(nc, in_maps, core_ids=list(range(8)))

    out = np.empty((B, L, H), dtype=np.float32)
    for core, r in enumerate(res.results):
        b, i = core // 4, core % 4
        out[b, i * L_CHUNK:(i + 1) * L_CHUNK] = r["y"]
    return out
